# revision 30
# baseline (speedup 1.0000x reference)
"""Trainium2 Bass kernel for nn_Cross_SA_Layer_Linear (sparse linear attention layer).

Sharding (8 cores):
- Phase A (k0/v0/softmax/kv for the 4 first-group batches): core c computes
  batch c%4 over the FULL position dim N=4096 locally (exact local softmax),
  then one AllGather of the bf16 kv[512,512]+ksum[512] payload over quad
  groups [[0,1,2,3],[4,5,6,7]] distributes all 4 batches to every core.
- Phases B (q/attention over the 12 'rest' batches) and C (LN+FFN on the 4
  output batches) are data-parallel over N: core c owns positions
  [c*512,(c+1)*512) of every batch; the mean over m is local per position.
- Only the first 4 output batches are computed; 'rest' passes through on host.
"""
import sys
sys.path.insert(0, '/opt/trn_rl_repo')
import numpy as np
import ml_dtypes

from concourse import bass, bacc, mybir, tile
from concourse import bass_utils
from concourse import bass_isa

f32 = mybir.dt.float32
bf16 = mybir.dt.bfloat16
AF = mybir.ActivationFunctionType
ALU = mybir.AluOpType
AX = mybir.AxisListType

NCORES = 8
GROUP, B, C, N = 4, 4, 512, 4096
M = (GROUP - 1) * B          # 12 rest batches
NL = N // NCORES             # 512 local positions
CT = C // 128                # 4 channel tiles
JT = (4 * C) // 128          # 16 hidden tiles
NSTRIPE = N // 128           # 32 full-N stripes (phase A)
EPS_LN, EPS_Z = 1e-6, 1e-6
PREFETCH_M = 4               # m's of q/pr emitted before AG-dependent work

_CACHE = {}


def _build(reps=1, parts=("A", "CC", "A2", "B", "C")):
    nc = bacc.Bacc("TRN2", target_bir_lowering=False, debug=False,
                   num_devices=NCORES)

    # ---- DRAM I/O ----
    xs_d = nc.dram_tensor("xs", [GROUP * B, C, NL], f32, kind="ExternalInput")
    xfull_d = nc.dram_tensor("xfull", [C, N], f32, kind="ExternalInput")
    wqT_d = nc.dram_tensor("wqT", [4, 128, 128], bf16, kind="ExternalInput")
    wkT_d = nc.dram_tensor("wkT", [4, 128, 128], bf16, kind="ExternalInput")
    wvT_d = nc.dram_tensor("wvT", [C, C], bf16, kind="ExternalInput")
    wphiT_d = nc.dram_tensor("wphiT", [C, C], bf16, kind="ExternalInput")
    fc1T_d = nc.dram_tensor("fc1T", [C, 4 * C], bf16, kind="ExternalInput")
    fc2T_d = nc.dram_tensor("fc2T", [4 * C, C], bf16, kind="ExternalInput")
    fc1b_d = nc.dram_tensor("fc1b", [128, JT], f32, kind="ExternalInput")
    fc2b_d = nc.dram_tensor("fc2b", [128, CT], f32, kind="ExternalInput")
    ln1w_d = nc.dram_tensor("ln1w", [128, CT], f32, kind="ExternalInput")
    ln1b_d = nc.dram_tensor("ln1b", [128, CT], f32, kind="ExternalInput")
    ln2w_d = nc.dram_tensor("ln2w", [128, CT], f32, kind="ExternalInput")
    ln2b_d = nc.dram_tensor("ln2b", [128, CT], f32, kind="ExternalInput")
    out_d = nc.dram_tensor("out", [B, C, NL], f32, kind="ExternalOutput")

    xs = xs_d.ap()
    xfull = xfull_d.ap()

    with tile.TileContext(nc) as tc:
        with (
            tc.tile_pool(name="wp", bufs=1) as wp,        # weights, whole kernel
            tc.tile_pool(name="pers", bufs=1) as pers,    # ys/kv/ks, phases B-C
            tc.tile_pool(name="dp", bufs=1, space="DRAM") as dp,
        ):
            # ---- constants / weights in SBUF ----
            ones_col = wp.tile([128, 1], bf16, name="ones_col")
            nc.gpsimd.memset(ones_col[:], 1.0)
            ones_row_f = wp.tile([1, 128], f32, name="ones_row_f")
            nc.gpsimd.memset(ones_row_f[:], 1.0)
            ones_row_b = wp.tile([1, 128], bf16, name="ones_row_b")
            nc.gpsimd.memset(ones_row_b[:], 1.0)
            eps_ln1 = wp.tile([1, 1], f32, name="eps_ln1")
            nc.gpsimd.memset(eps_ln1[:], EPS_LN)

            wq = [wp.tile([128, 128], bf16, name=f"wq{g}") for g in range(4)]
            wk = [wp.tile([128, 128], bf16, name=f"wk{g}") for g in range(4)]
            for g in range(4):
                nc.sync.dma_start(wq[g][:], wqT_d.ap()[g])
                nc.sync.dma_start(wk[g][:], wkT_d.ap()[g])
            wv = [wp.tile([128, C], bf16, name=f"wv{k}") for k in range(CT)]
            wphi = [wp.tile([128, C], bf16, name=f"wphi{k}") for k in range(CT)]
            for k in range(CT):
                nc.sync.dma_start(wv[k][:], wvT_d.ap()[k * 128:(k + 1) * 128, :])
                nc.sync.dma_start(wphi[k][:], wphiT_d.ap()[k * 128:(k + 1) * 128, :])
            fc1 = [wp.tile([128, 4 * C], bf16, name=f"fc1_{k}") for k in range(CT)]
            for k in range(CT):
                nc.sync.dma_start(fc1[k][:], fc1T_d.ap()[k * 128:(k + 1) * 128, :])
            fc2 = [wp.tile([128, C], bf16, name=f"fc2_{k}") for k in range(JT)]
            for k in range(JT):
                nc.sync.dma_start(fc2[k][:], fc2T_d.ap()[k * 128:(k + 1) * 128, :])
            fc1b = wp.tile([128, JT], f32, name="fc1b")
            fc2b = wp.tile([128, CT], f32, name="fc2b")
            ln1w = wp.tile([128, CT], f32, name="ln1w")
            ln1b = wp.tile([128, CT], f32, name="ln1b")
            ln2w = wp.tile([128, CT], f32, name="ln2w")
            ln2b = wp.tile([128, CT], f32, name="ln2b")
            for t, d in [(fc1b, fc1b_d), (fc2b, fc2b_d), (ln1w, ln1w_d),
                         (ln1b, ln1b_d), (ln2w, ln2w_d), (ln2b, ln2b_d)]:
                nc.sync.dma_start(t[:], d.ap()[:])

            # ---- DRAM scratch ----
            ag_in = dp.tile([C + 1, C], bf16, name="ag_in")
            ag_out = dp.tile([B * (C + 1), C], bf16, name="ag_out")

            # persistent activation state (B->C)
            ysum = [pers.tile([128, NL], f32, name=f"ys{k}") for k in range(CT)]
            phi1 = [[pers.tile([128, NL], f32, name=f"phi{b}_{d}")
                     for d in range(CT)] for b in range(B)]
            kvb = [[pers.tile([128, C], bf16, name=f"kvb{b}_{k}") for k in range(CT)]
                   for b in range(B)]
            kscol = [[pers.tile([128, 1], bf16, name=f"ksc{b}_{k}") for k in range(CT)]
                     for b in range(B)]

            for _rep in range(reps):
                for k in range(CT):
                    nc.vector.memset(ysum[k][:], 0.0)
                # =================================================================
                # PHASE A: full-N attention stats for batch bA = core%4 (the
                # per-core xfull input selects the batch; program is identical).
                # =================================================================
                with (
                    tc.tile_pool(name="pA", bufs=1) as pA,
                ):
                  if "A" in parts:
                    k0p = [pA.tile([128, C], bf16, name=f"k0p{i}")
                           for i in range(NSTRIPE)]
                    xbf = [pA.tile([128, N], bf16, name=f"xbf{k}")
                           for k in range(CT)]
                    Qg_row = pA.tile([1, C], f32, name="Qg_row")
                    NH = N // 2
                    for k in range(CT):
                        for h in range(2):
                            xf = pA.tile([128, NH], f32, name="xf", tag="xf",
                                         bufs=2)
                            nc.sync.dma_start(
                                xf[:], xfull[k * 128:(k + 1) * 128,
                                             h * NH:(h + 1) * NH])
                            nc.scalar.copy(xbf[k][:, h * NH:(h + 1) * NH], xf[:])
                    with tc.tile_pool(name="psA", bufs=1, space="PSUM") as psA:
                        psum_S = psA.tile([1, C], f32, name="pS", tag="pS", bufs=1)
                        for i in range(NSTRIPE):
                            sl = slice(i * 128, (i + 1) * 128)
                            pk = psA.tile([128, C], f32, name="pk", tag="pk", bufs=2)
                            for g in range(4):
                                nc.tensor.matmul(pk[:, g * 128:(g + 1) * 128],
                                                 xbf[g][:, sl], wk[g][:], start=True,
                                                 stop=True)
                            nc.scalar.activation(k0p[i][:], pk[:], AF.Relu)
                            nc.tensor.matmul(psum_S[:], ones_col[:], k0p[i][:],
                                             start=(i == 0), stop=(i == NSTRIPE - 1))

                        # Qg = S'/N + 1 (mean of k0'+1)
                        nc.scalar.copy(Qg_row[:], psum_S[:])
                        nc.vector.tensor_scalar(Qg_row[:], Qg_row[:], 1.0 / N, 1.0,
                                                ALU.mult, ALU.add)
                        # broadcast Qg to all partitions via PE
                        qgb = psA.tile([128, C], f32, name="qgb", tag="qgb", bufs=1)
                        nc.tensor.matmul(qgb[:], ones_row_f[:], Qg_row[:],
                                         start=True, stop=True)
                        Qg_bc = pA.tile([128, C], f32, name="Qg_bc")
                        nc.scalar.copy(Qg_bc[:], qgb[:])

                        # t[n] = sum_c Qg*k0' per stripe
                        tall = pA.tile([128, NSTRIPE], f32, name="tall")
                        for i in range(NSTRIPE):
                            sc = pA.tile([128, C], f32, name="scA", tag="scA",
                                         bufs=2)
                            nc.vector.tensor_tensor(sc[:], k0p[i][:], Qg_bc[:],
                                                    ALU.mult)
                            nc.vector.tensor_reduce(tall[:, i:i + 1], sc[:], AX.X,
                                                    ALU.add)
                    rmaxc = pA.tile([128, 1], f32, name="rmaxc")
                    nc.vector.tensor_reduce(rmaxc[:], tall[:], AX.X, ALU.max)
                    mx11 = pA.tile([1, 1], f32, name="mx11")
                    nc.gpsimd.tensor_reduce(mx11[:], rmaxc[:], AX.C, ALU.max)
                    negmx = pA.tile([1, 1], f32, name="negmx")
                    nc.vector.tensor_scalar(negmx[:], mx11[:], -1.0, None, ALU.mult)
                    negmx_col = pA.tile([128, 1], f32, name="negmx_col")
                    nc.gpsimd.partition_broadcast(negmx_col[:], negmx[:])

                    # e = exp(t-max); khat = (k0'+1)*e; kv += khat^T@v0; ks += sum
                    with tc.tile_pool(name="psKV", bufs=1, space="PSUM") as psKV:
                        psum_kv = [psKV.tile([128, C], f32, name=f"pkv{k}",
                                             tag=f"pkv{k}", bufs=1) for k in range(CT)]
                        psum_ks = psKV.tile([1, C], f32, name="pks", tag="pks", bufs=1)
                        e_tall = pA.tile([128, NSTRIPE], f32, name="e_tall")
                        for i in range(NSTRIPE):
                            sl = slice(i * 128, (i + 1) * 128)
                            nc.scalar.activation(e_tall[:, i:i + 1], tall[:, i:i + 1],
                                                 AF.Exp, bias=negmx_col[:])
                            pv = psKV.tile([128, C], f32, name="pv", tag="pv",
                                           bufs=2)
                            for k in range(CT):
                                nc.tensor.matmul(pv[:], xbf[k][:, sl], wv[k][:],
                                                 start=(k == 0), stop=(k == CT - 1))
                            v0i = pA.tile([128, C], bf16, name="v0i", tag="v0i",
                                          bufs=3)
                            nc.scalar.copy(v0i[:], pv[:])
                            kh = pA.tile([128, C], bf16, name="khat", tag="khat",
                                         bufs=4)
                            nc.vector.tensor_scalar(kh[:], k0p[i][:], 1.0,
                                                    e_tall[:, i:i + 1],
                                                    ALU.add, ALU.mult)
                            for k in range(CT):
                                nc.tensor.matmul(psum_kv[k][:],
                                                 kh[:, k * 128:(k + 1) * 128],
                                                 v0i[:], start=(i == 0),
                                                 stop=(i == NSTRIPE - 1))
                            nc.tensor.matmul(psum_ks[:], ones_col[:], kh[:],
                                             start=(i == 0), stop=(i == NSTRIPE - 1))
                        rsec = pA.tile([128, 1], f32, name="rsec")
                        nc.vector.tensor_reduce(rsec[:], e_tall[:], AX.X, ALU.add)
                        se11 = pA.tile([1, 1], f32, name="se11")
                        nc.gpsimd.tensor_reduce(se11[:], rsec[:], AX.C, ALU.add)
                        # alpha = N/se (ksum scale), beta = N/(se*sqrt(C)) (kv scale)
                        alpha11 = pA.tile([1, 1], f32, name="alpha11")
                        nc.vector.reciprocal(alpha11[:], se11[:])
                        nc.vector.tensor_scalar(alpha11[:], alpha11[:], float(N),
                                                None, ALU.mult)
                        acol = pA.tile([128, 1], f32, name="acol")
                        nc.gpsimd.partition_broadcast(acol[:], alpha11[:])
                        alpha_col = acol
                        beta_col = pA.tile([128, 1], f32, name="beta_col")
                        nc.vector.tensor_scalar(beta_col[:], alpha_col[:],
                                                1.0 / float(np.sqrt(C)), None,
                                                ALU.mult)
                        for k in range(CT):
                            stg = pA.tile([128, C], bf16, name="kvstg", tag="kvstg",
                                          bufs=2)
                            nc.vector.tensor_scalar(stg[:], psum_kv[k][:],
                                                    beta_col[:], None, ALU.mult)
                            nc.sync.dma_start(ag_in[:][k * 128:(k + 1) * 128, :],
                                              stg[:])
                        ks_bf = pA.tile([1, C], bf16, name="ks_bf")
                        nc.vector.tensor_scalar(ks_bf[:], psum_ks[:],
                                                alpha_col[0:1, :], None, ALU.mult)
                        nc.sync.dma_start(ag_in[:][C:C + 1, :], ks_bf[:])

                  if "CC" in parts:
                    nc.gpsimd.collective_compute(
                        "AllGather", ALU.bypass,
                        replica_groups=[[0, 1, 2, 3], [4, 5, 6, 7]],
                        ins=[ag_in[:]], outs=[ag_out[:]],
                    )

                # =================================================================
                # PHASE A': phi_first (N-shard) -> DRAM spill  (covers AG latency)
                # =================================================================
                with (
                    tc.tile_pool(name="pA2", bufs=1) as pA2,
                    tc.tile_pool(name="psA2", bufs=1, space="PSUM") as psA2,
                ):
                  if "A2" in parts:
                    for b in range(B):
                        ff = [pA2.tile([128, NL], f32, name="ff", tag="ff", bufs=8)
                              for _ in range(CT)]
                        fbf = [pA2.tile([128, NL], bf16, name="fbf", tag="fbf",
                                        bufs=8) for _ in range(CT)]
                        for k in range(CT):
                            nc.sync.dma_start(ff[k][:],
                                              xs[b, k * 128:(k + 1) * 128, :])
                            nc.vector.tensor_copy(fbf[k][:], ff[k][:])
                        for d in range(CT):
                            pf = psA2.tile([128, NL], f32, name="ppf", tag="ppf",
                                           bufs=2)
                            for k in range(CT):
                                nc.tensor.matmul(pf[:],
                                                 wphi[k][:, d * 128:(d + 1) * 128],
                                                 fbf[k][:], start=(k == 0),
                                                 stop=(k == CT - 1))
                            nc.scalar.activation(phi1[b][d][:], pf[:], AF.Copy,
                                                 scale=1.0 / M)

                # =================================================================
                # PHASE B: q/attention for 12 rest batches on local positions
                # =================================================================
                with (
                    tc.tile_pool(name="pB", bufs=1) as pB,
                    tc.tile_pool(name="psB", bufs=1, space="PSUM") as psB,
                ):
                  if "B" in parts:
                    qbuf = {}

                    def emit_qpr(m):
                        rf = [pB.tile([128, NL], f32, name="rf", tag="rf", bufs=8)
                              for _ in range(CT)]
                        rb = [pB.tile([128, NL], bf16, name="rb", tag="rb", bufs=8)
                              for _ in range(CT)]
                        for k in range(CT):
                            nc.sync.dma_start(rf[k][:],
                                              xs[B + m, k * 128:(k + 1) * 128, :])
                            nc.scalar.copy(rb[k][:], rf[k][:])
                        qb, pb = [], []
                        for g in range(4):
                            pq = psB.tile([128, NL], f32, name="pq", tag="pq", bufs=1)
                            nc.tensor.matmul(pq[:], wq[g][:], rb[g][:], start=True,
                                             stop=True)
                            q = pB.tile([128, NL], bf16, name="qbf", tag="qbf",
                                        bufs=4 * (PREFETCH_M + 2))
                            nc.vector.tensor_scalar(q[:], pq[:], 0.0, 1.0, ALU.max,
                                                    ALU.add)
                            qb.append(q)
                        for d in range(CT):
                            pr = psB.tile([128, NL], f32, name="ppr", tag="ppr",
                                          bufs=2)
                            for k in range(CT):
                                nc.tensor.matmul(pr[:],
                                                 wphi[k][:, d * 128:(d + 1) * 128],
                                                 rb[k][:], start=(k == 0),
                                                 stop=(k == CT - 1))
                            p = pB.tile([128, NL], bf16, name="prbf", tag="prbf",
                                        bufs=4 * (PREFETCH_M + 2))
                            nc.scalar.copy(p[:], pr[:])
                            pb.append(p)
                        qbuf[m] = (qb, pb)

                    def emit_attn(m):
                        b = m % B
                        qb, pb = qbuf.pop(m)
                        pz = psB.tile([1, NL], f32, name="pz", tag="pz", bufs=1)
                        for k in range(CT):
                            nc.tensor.matmul(pz[:], kscol[b][k][:], qb[k][:],
                                             start=(k == 0), stop=(k == CT - 1))
                        zrow = pB.tile([1, NL], f32, name="zrow", tag="zrow", bufs=2)
                        nc.vector.tensor_scalar(zrow[:], pz[:], EPS_Z, None, ALU.add)
                        nc.vector.reciprocal(zrow[:], zrow[:])
                        zrow_bf = pB.tile([1, NL], bf16, name="zrow_bf",
                                          tag="zrow_bf", bufs=2)
                        nc.scalar.copy(zrow_bf[:], zrow[:])
                        zb_ps = psB.tile([128, NL], f32, name="zb_ps", tag="zb_ps",
                                         bufs=1)
                        nc.tensor.matmul(zb_ps[:], ones_row_b[:], zrow_bf[:],
                                         start=True, stop=True)
                        zbc = pB.tile([128, NL], bf16, name="zbc", tag="zbc", bufs=2)
                        nc.scalar.copy(zbc[:], zb_ps[:])
                        qz = []
                        for k in range(CT):
                            t = pB.tile([128, NL], bf16, name="qz", tag="qz", bufs=8)
                            nc.vector.tensor_tensor(t[:], qb[k][:], zbc[:], ALU.mult)
                            qz.append(t)
                        for d in range(CT):
                            sp = psB.tile([128, NL], f32, name="psmm", tag="psmm",
                                          bufs=2)
                            for k in range(CT):
                                nc.tensor.matmul(sp[:],
                                                 kvb[b][k][:, d * 128:(d + 1) * 128],
                                                 qz[k][:], start=(k == 0),
                                                 stop=(k == CT - 1))
                            tmp = pB.tile([128, NL], f32, name="ytmp", tag="ytmp",
                                          bufs=2)
                            nc.vector.tensor_tensor(tmp[:], sp[:], pb[d][:], ALU.mult)
                            nc.vector.tensor_tensor(ysum[d][:], ysum[d][:], tmp[:],
                                                    ALU.add)

                    for m in range(PREFETCH_M):
                        emit_qpr(m)
                    # post-AG loads (kv + ksum for all 4 batches)
                    ago = ag_out[:]
                    for b in range(B):
                        for k in range(CT):
                            nc.sync.dma_start(
                                kvb[b][k][:],
                                ago[b * (C + 1) + k * 128:
                                    b * (C + 1) + (k + 1) * 128, :])
                            nc.sync.dma_start(
                                kscol[b][k][:],
                                ago[b * (C + 1) + C: b * (C + 1) + C + 1,
                                    k * 128:(k + 1) * 128].rearrange("a b -> b a"))
                    for m in range(M):
                        if m >= PREFETCH_M:
                            emit_qpr(m)
                        emit_attn(m)

                # =================================================================
                # PHASE C: y_first, LN1, FFN, LN2, relu  (per output batch b)
                # =================================================================
                with (
                    tc.tile_pool(name="pC", bufs=1) as pC,
                    tc.tile_pool(name="psC", bufs=1, space="PSUM") as psC,
                ):
                  if "C" in parts:
                    for b in range(B):
                        fst = [pC.tile([128, NL], f32, name="fst", tag="fst", bufs=6)
                               for _ in range(CT)]
                        for k in range(CT):
                            nc.sync.dma_start(fst[k][:],
                                              xs[b, k * 128:(k + 1) * 128, :])
                        res1 = [pC.tile([128, NL], f32, name=f"res1_{k}",
                                        tag=f"res1_{k}", bufs=1) for k in range(CT)]
                        for k in range(CT):
                            nc.vector.tensor_tensor(res1[k][:], ysum[k][:],
                                                    phi1[b][k][:], ALU.mult)
                            nc.vector.tensor_tensor(res1[k][:], res1[k][:],
                                                    fst[k][:], ALU.add)

                        def layer_norm(xtiles, out_cb):
                            # stats over channel (partition) axis via PE ones-matmul
                            xbt = [pC.tile([128, NL], bf16, name="lnxb", tag="lnxb",
                                           bufs=8) for _ in range(CT)]
                            sqt = [pC.tile([128, NL], bf16, name="lnsqb",
                                           tag="lnsqb", bufs=8) for _ in range(CT)]
                            for k in range(CT):
                                nc.scalar.copy(xbt[k][:], xtiles[k][:])
                                nc.scalar.activation(sqt[k][:], xtiles[k][:],
                                                     AF.Square)
                            ps_st = psC.tile([1, NL], f32, name="lnst", tag="lnst",
                                             bufs=1)
                            for k in range(CT):
                                nc.tensor.matmul(ps_st[:], ones_col[:], xbt[k][:],
                                                 start=(k == 0), stop=(k == CT - 1))
                            mu = pC.tile([1, NL], f32, name="lnmu", tag="lnmu",
                                         bufs=1)
                            nc.scalar.activation(mu[:], ps_st[:], AF.Copy,
                                                 scale=1.0 / C)
                            ps_sq = psC.tile([1, NL], f32, name="lnst", tag="lnst",
                                             bufs=1)
                            for k in range(CT):
                                nc.tensor.matmul(ps_sq[:], ones_col[:], sqt[k][:],
                                                 start=(k == 0), stop=(k == CT - 1))
                            musq = pC.tile([1, NL], f32, name="lnmusq", tag="lnmusq",
                                           bufs=1)
                            nc.scalar.activation(musq[:], mu[:], AF.Square)
                            var = pC.tile([1, NL], f32, name="lnvar", tag="lnvar",
                                          bufs=1)
                            nc.vector.scalar_tensor_tensor(
                                var[:], ps_sq[:], 1.0 / C, musq[:],
                                ALU.mult, ALU.subtract)
                            sd = pC.tile([1, NL], f32, name="lnsd", tag="lnsd",
                                         bufs=1)
                            nc.scalar.activation(sd[:], var[:], AF.Sqrt,
                                                 bias=eps_ln1[:])
                            arow = pC.tile([1, NL], f32, name="lnA", tag="lnA",
                                           bufs=1)
                            nc.vector.reciprocal(arow[:], sd[:])
                            brow = pC.tile([1, NL], f32, name="lnB", tag="lnB",
                                           bufs=1)
                            nc.vector.scalar_tensor_tensor(
                                brow[:], mu[:], -1.0, arow[:], ALU.mult, ALU.mult)
                            # broadcast a/b rows to 128 partitions via PE (f32)
                            ps_a = psC.tile([128, NL], f32, name="lnbc", tag="lnbc",
                                            bufs=1)
                            nc.tensor.matmul(ps_a[:], ones_row_f[:], arow[:],
                                             start=True, stop=True)
                            abc = pC.tile([128, NL], f32, name="lnAbc", tag="lnAbc",
                                          bufs=1)
                            nc.scalar.copy(abc[:], ps_a[:])
                            ps_b = psC.tile([128, NL], f32, name="lnbc", tag="lnbc",
                                            bufs=1)
                            nc.tensor.matmul(ps_b[:], ones_row_f[:], brow[:],
                                             start=True, stop=True)
                            bbc = pC.tile([128, NL], f32, name="lnBbc", tag="lnBbc",
                                          bufs=1)
                            nc.scalar.copy(bbc[:], ps_b[:])
                            for k in range(CT):
                                t = pC.tile([128, NL], f32, name="lnt", tag="lnt",
                                            bufs=2)
                                nc.vector.tensor_tensor(t[:], xtiles[k][:], abc[:],
                                                        ALU.mult)
                                nc.vector.tensor_tensor(t[:], t[:], bbc[:], ALU.add)
                                out_cb(k, t)

                        o1 = [pC.tile([128, NL], f32, name=f"o1_{k}",
                                      tag=f"o1_{k}", bufs=1) for k in range(CT)]
                        o1b = [pC.tile([128, NL], bf16, name=f"o1b_{k}",
                                       tag=f"o1b_{k}", bufs=1) for k in range(CT)]

                        def ln1_out(k, t):
                            nc.scalar.activation(o1[k][:], t[:], AF.Identity,
                                                 bias=ln1b[:, k:k + 1],
                                                 scale=ln1w[:, k:k + 1])
                            nc.vector.tensor_copy(o1b[k][:], o1[k][:])

                        layer_norm(res1, ln1_out)

                        # FFN: h = relu(fc1@o1+b1) [j,n]; o = fc2@h [d,n]
                        po = [psC.tile([128, NL], f32, name=f"po{d}", tag=f"po{d}",
                                       bufs=1) for d in range(CT)]
                        for j in range(JT):
                            ph = psC.tile([128, NL], f32, name="ph", tag="ph",
                                          bufs=2)
                            for k in range(CT):
                                nc.tensor.matmul(ph[:],
                                                 fc1[k][:, j * 128:(j + 1) * 128],
                                                 o1b[k][:], start=(k == 0),
                                                 stop=(k == CT - 1))
                            hb = pC.tile([128, NL], bf16, name="hbf", tag="hbf",
                                         bufs=3)
                            nc.scalar.activation(hb[:], ph[:], AF.Relu,
                                                 bias=fc1b[:, j:j + 1])
                            for d in range(CT):
                                nc.tensor.matmul(po[d][:],
                                                 fc2[j][:, d * 128:(d + 1) * 128],
                                                 hb[:], start=(j == 0),
                                                 stop=(j == JT - 1))
                        o2 = [pC.tile([128, NL], f32, name=f"o2_{k}",
                                      tag=f"o2_{k}", bufs=1) for k in range(CT)]
                        for d in range(CT):
                            nc.scalar.activation(o2[d][:], po[d][:], AF.Identity,
                                                 bias=fc2b[:, d:d + 1])
                            nc.vector.tensor_tensor(o2[d][:], o2[d][:], o1[d][:],
                                                    ALU.add)

                        def ln2_out(k, t):
                            ot = pC.tile([128, NL], f32, name="otile", tag="otile",
                                         bufs=4)
                            nc.scalar.activation(ot[:], t[:], AF.Relu,
                                                 bias=ln2b[:, k:k + 1],
                                                 scale=ln2w[:, k:k + 1])
                            nc.sync.dma_start(
                                out_d.ap()[b, k * 128:(k + 1) * 128, :], ot[:])

                        layer_norm(o2, ln2_out)

    nc.compile()
    return nc


def _prep_inputs(inputs):
    x = np.asarray(inputs['x'], np.float32)
    bf = ml_dtypes.bfloat16

    def col(a):   # [k*128] -> [128, k]
        a = np.asarray(a, np.float32)
        return np.ascontiguousarray(a.reshape(-1, 128).T)

    shared = {
        "wqT": np.ascontiguousarray(
            np.asarray(inputs['Wq'], np.float32).transpose(0, 2, 1)).astype(bf),
        "wkT": np.ascontiguousarray(
            np.asarray(inputs['Wk'], np.float32).transpose(0, 2, 1)).astype(bf),
        "wvT": np.ascontiguousarray(np.asarray(inputs['Wv'], np.float32).T).astype(bf),
        "wphiT": np.ascontiguousarray(
            np.asarray(inputs['Wphi'], np.float32).T).astype(bf),
        "fc1T": np.ascontiguousarray(
            np.asarray(inputs['fc1_w'], np.float32).T).astype(bf),
        "fc2T": np.ascontiguousarray(
            np.asarray(inputs['fc2_w'], np.float32).T).astype(bf),
        "fc1b": col(inputs['fc1_b']),
        "fc2b": col(inputs['fc2_b']),
        "ln1w": col(inputs['ln1_w']),
        "ln1b": col(inputs['ln1_b']),
        "ln2w": col(inputs['ln2_w']),
        "ln2b": col(inputs['ln2_b']),
    }
    in_maps = []
    for c in range(NCORES):
        m = dict(shared)
        m["xs"] = np.ascontiguousarray(x[:, :, c * NL:(c + 1) * NL])
        m["xfull"] = np.ascontiguousarray(x[c % B])
        in_maps.append(m)
    return in_maps


def kernel(**inputs):
    if "nc" not in _CACHE:
        _CACHE["nc"] = _build()
    nc = _CACHE["nc"]
    in_maps = _prep_inputs(inputs)
    r = bass_utils.run_bass_kernel_spmd(nc, in_maps, core_ids=list(range(NCORES)))
    x = np.asarray(inputs['x'], np.float32)
    out_first = np.empty((B, C, N), np.float32)
    for c in range(NCORES):
        out_first[:, :, c * NL:(c + 1) * NL] = r.results[c]["out"]
    return np.concatenate([out_first, x[B:]], axis=0)



# revision 33
# speedup vs baseline: 3.5129x; 3.5129x over previous
"""Trainium2 Bass kernel for nn_Cross_SA_Layer_Linear (sparse linear attention layer).

Sharding (8 cores):
- Phase A (k0/v0/softmax/kv for the 4 first-group batches): core c computes
  batch c%4 over the FULL position dim N=4096 locally (exact local softmax),
  then one AllGather of the bf16 kv[512,512]+ksum[512] payload over quad
  groups [[0,1,2,3],[4,5,6,7]] distributes all 4 batches to every core.
- Phases B (q/attention over the 12 'rest' batches) and C (LN+FFN on the 4
  output batches) are data-parallel over N: core c owns positions
  [c*512,(c+1)*512) of every batch; the mean over m is local per position.
- Only the first 4 output batches are computed; 'rest' passes through on host.
"""
import sys
sys.path.insert(0, '/opt/trn_rl_repo')
import numpy as np
import ml_dtypes

from concourse import bass, bacc, mybir, tile
from concourse import bass_utils
from concourse import bass_isa

f32 = mybir.dt.float32
bf16 = mybir.dt.bfloat16
AF = mybir.ActivationFunctionType
ALU = mybir.AluOpType
AX = mybir.AxisListType

NCORES = 8
GROUP, B, C, N = 4, 4, 512, 4096
M = (GROUP - 1) * B          # 12 rest batches
NL = N // NCORES             # 512 local positions
CT = C // 128                # 4 channel tiles
JT = (4 * C) // 128          # 16 hidden tiles
NSTRIPE = N // 128           # 32 full-N stripes (phase A)
EPS_LN, EPS_Z = 1e-6, 1e-6
PREFETCH_M = 4               # m's of q/pr emitted before AG-dependent work

_CACHE = {}
_PROBE_NOQ7 = False  # timing-only probe: replace tiny gpsimd ops with memsets


def _build(reps=1, parts=("A", "CC", "A2", "B", "C")):
    nc = bacc.Bacc("TRN2", target_bir_lowering=False, debug=False,
                   num_devices=NCORES)

    # ---- DRAM I/O ----
    xs_d = nc.dram_tensor("xs", [GROUP * B, C, NL], f32, kind="ExternalInput")
    xfull_d = nc.dram_tensor("xfull", [C, N], f32, kind="ExternalInput")
    wqT_d = nc.dram_tensor("wqT", [4, 128, 128], bf16, kind="ExternalInput")
    wkT_d = nc.dram_tensor("wkT", [4, 128, 128], bf16, kind="ExternalInput")
    wvT_d = nc.dram_tensor("wvT", [C, C], bf16, kind="ExternalInput")
    wphiT_d = nc.dram_tensor("wphiT", [C, C], bf16, kind="ExternalInput")
    fc1T_d = nc.dram_tensor("fc1T", [C, 4 * C], bf16, kind="ExternalInput")
    fc2T_d = nc.dram_tensor("fc2T", [4 * C, C], bf16, kind="ExternalInput")
    fc1b_d = nc.dram_tensor("fc1b", [128, JT], f32, kind="ExternalInput")
    fc2b_d = nc.dram_tensor("fc2b", [128, CT], f32, kind="ExternalInput")
    ln1w_d = nc.dram_tensor("ln1w", [128, CT], f32, kind="ExternalInput")
    ln1b_d = nc.dram_tensor("ln1b", [128, CT], f32, kind="ExternalInput")
    ln2w_d = nc.dram_tensor("ln2w", [128, CT], f32, kind="ExternalInput")
    ln2b_d = nc.dram_tensor("ln2b", [128, CT], f32, kind="ExternalInput")
    out_d = nc.dram_tensor("out", [B, C, NL], f32, kind="ExternalOutput")

    xs = xs_d.ap()
    xfull = xfull_d.ap()

    with tile.TileContext(nc) as tc:
        with (
            tc.tile_pool(name="wp", bufs=1) as wp,        # weights, whole kernel
            tc.tile_pool(name="pers", bufs=1) as pers,    # ys/kv/ks, phases B-C
            tc.tile_pool(name="dp", bufs=1, space="DRAM") as dp,
        ):
            # ---- constants / weights in SBUF ----
            ones_col = wp.tile([128, 1], bf16, name="ones_col")
            nc.gpsimd.memset(ones_col[:], 1.0)
            ones_row_f = wp.tile([1, 128], f32, name="ones_row_f")
            nc.gpsimd.memset(ones_row_f[:], 1.0)
            ones_row_b = wp.tile([1, 128], bf16, name="ones_row_b")
            nc.gpsimd.memset(ones_row_b[:], 1.0)
            eps_ln1 = wp.tile([1, 1], f32, name="eps_ln1")
            nc.gpsimd.memset(eps_ln1[:], EPS_LN)

            wq = [wp.tile([128, 128], bf16, name=f"wq{g}") for g in range(4)]
            wk = [wp.tile([128, 128], bf16, name=f"wk{g}") for g in range(4)]
            for g in range(4):
                nc.sync.dma_start(wq[g][:], wqT_d.ap()[g])
                nc.sync.dma_start(wk[g][:], wkT_d.ap()[g])
            wv = [wp.tile([128, C], bf16, name=f"wv{k}") for k in range(CT)]
            wphi = [wp.tile([128, C], bf16, name=f"wphi{k}") for k in range(CT)]
            for k in range(CT):
                nc.sync.dma_start(wv[k][:], wvT_d.ap()[k * 128:(k + 1) * 128, :])
                nc.sync.dma_start(wphi[k][:], wphiT_d.ap()[k * 128:(k + 1) * 128, :])
            fc1 = [wp.tile([128, 4 * C], bf16, name=f"fc1_{k}") for k in range(CT)]
            for k in range(CT):
                nc.sync.dma_start(fc1[k][:], fc1T_d.ap()[k * 128:(k + 1) * 128, :])
            fc2 = [wp.tile([128, C], bf16, name=f"fc2_{k}") for k in range(JT)]
            for k in range(JT):
                nc.sync.dma_start(fc2[k][:], fc2T_d.ap()[k * 128:(k + 1) * 128, :])
            fc1b = wp.tile([128, JT], f32, name="fc1b")
            fc2b = wp.tile([128, CT], f32, name="fc2b")
            ln1w = wp.tile([128, CT], f32, name="ln1w")
            ln1b = wp.tile([128, CT], f32, name="ln1b")
            ln2w = wp.tile([128, CT], f32, name="ln2w")
            ln2b = wp.tile([128, CT], f32, name="ln2b")
            for t, d in [(fc1b, fc1b_d), (fc2b, fc2b_d), (ln1w, ln1w_d),
                         (ln1b, ln1b_d), (ln2w, ln2w_d), (ln2b, ln2b_d)]:
                nc.sync.dma_start(t[:], d.ap()[:])

            # ---- DRAM scratch ----
            ag_in = dp.tile([C + 1, C], bf16, name="ag_in")
            ag_out = dp.tile([B * (C + 1), C], bf16, name="ag_out")

            # persistent activation state (B->C)
            ysum = [pers.tile([128, NL], f32, name=f"ys{k}") for k in range(CT)]
            phi1 = [[pers.tile([128, NL], f32, name=f"phi{b}_{d}")
                     for d in range(CT)] for b in range(B)]
            kvb = [[pers.tile([128, C], bf16, name=f"kvb{b}_{k}") for k in range(CT)]
                   for b in range(B)]
            kscol = [[pers.tile([128, 1], bf16, name=f"ksc{b}_{k}") for k in range(CT)]
                     for b in range(B)]

            for _rep in range(reps):
                for k in range(CT):
                    nc.vector.memset(ysum[k][:], 0.0)
                # =================================================================
                # PHASE A: full-N attention stats for batch bA = core%4 (the
                # per-core xfull input selects the batch; program is identical).
                # =================================================================
                with (
                    tc.tile_pool(name="pA", bufs=1) as pA,
                ):
                  if "A" in parts:
                    k0p = [pA.tile([128, C], bf16, name=f"k0p{i}")
                           for i in range(NSTRIPE)]
                    xbf = [pA.tile([128, N], bf16, name=f"xbf{k}")
                           for k in range(CT)]
                    Qg_row = pA.tile([1, C], f32, name="Qg_row")
                    NH = N // 2
                    for k in range(CT):
                        for h in range(2):
                            xf = pA.tile([128, NH], f32, name="xf", tag="xf",
                                         bufs=2)
                            nc.sync.dma_start(
                                xf[:], xfull[k * 128:(k + 1) * 128,
                                             h * NH:(h + 1) * NH])
                            nc.scalar.copy(xbf[k][:, h * NH:(h + 1) * NH], xf[:])
                    with tc.tile_pool(name="psA", bufs=1, space="PSUM") as psA:
                        psum_S = psA.tile([1, C], f32, name="pS", tag="pS", bufs=1)
                        for i in range(NSTRIPE):
                            sl = slice(i * 128, (i + 1) * 128)
                            pk = psA.tile([128, C], f32, name="pk", tag="pk", bufs=2)
                            for g in range(4):
                                nc.tensor.matmul(pk[:, g * 128:(g + 1) * 128],
                                                 xbf[g][:, sl], wk[g][:], start=True,
                                                 stop=True)
                            nc.scalar.activation(k0p[i][:], pk[:], AF.Relu)
                            nc.tensor.matmul(psum_S[:], ones_col[:], k0p[i][:],
                                             start=(i == 0), stop=(i == NSTRIPE - 1))

                        # Qg = S'/N + 1 (mean of k0'+1)
                        nc.scalar.copy(Qg_row[:], psum_S[:])
                        nc.vector.tensor_scalar(Qg_row[:], Qg_row[:], 1.0 / N, 1.0,
                                                ALU.mult, ALU.add)
                        # broadcast Qg to all partitions via PE
                        qgb = psA.tile([128, C], f32, name="qgb", tag="qgb", bufs=1)
                        nc.tensor.matmul(qgb[:], ones_row_f[:], Qg_row[:],
                                         start=True, stop=True)
                        Qg_bc = pA.tile([128, C], f32, name="Qg_bc")
                        nc.scalar.copy(Qg_bc[:], qgb[:])

                        # t[n] = sum_c Qg*k0' per stripe
                        tall = pA.tile([128, NSTRIPE], f32, name="tall")
                        for i in range(NSTRIPE):
                            sc = pA.tile([128, C], f32, name="scA", tag="scA",
                                         bufs=2)
                            nc.vector.tensor_tensor(sc[:], k0p[i][:], Qg_bc[:],
                                                    ALU.mult)
                            nc.vector.tensor_reduce(tall[:, i:i + 1], sc[:], AX.X,
                                                    ALU.add)
                    rmaxc = pA.tile([128, 1], f32, name="rmaxc")
                    nc.vector.tensor_reduce(rmaxc[:], tall[:], AX.X, ALU.max)
                    negmx_col = pA.tile([128, 1], f32, name="negmx_col")
                    if _PROBE_NOQ7:
                        nc.vector.memset(negmx_col[:], -100.0)
                    else:
                        mx11 = pA.tile([1, 1], f32, name="mx11")
                        nc.gpsimd.tensor_reduce(mx11[:], rmaxc[:], AX.C, ALU.max)
                        negmx = pA.tile([1, 1], f32, name="negmx")
                        nc.vector.tensor_scalar(negmx[:], mx11[:], -1.0, None,
                                                ALU.mult)
                        nc.gpsimd.partition_broadcast(negmx_col[:], negmx[:])

                    # e = exp(t-max); khat = (k0'+1)*e; kv += khat^T@v0; ks += sum
                    with tc.tile_pool(name="psKV", bufs=1, space="PSUM") as psKV:
                        psum_kv = [psKV.tile([128, C], f32, name=f"pkv{k}",
                                             tag=f"pkv{k}", bufs=1) for k in range(CT)]
                        psum_ks = psKV.tile([1, C], f32, name="pks", tag="pks", bufs=1)
                        e_tall = pA.tile([128, NSTRIPE], f32, name="e_tall")
                        for i in range(NSTRIPE):
                            sl = slice(i * 128, (i + 1) * 128)
                            nc.scalar.activation(e_tall[:, i:i + 1], tall[:, i:i + 1],
                                                 AF.Exp, bias=negmx_col[:])
                            pv = psKV.tile([128, C], f32, name="pv", tag="pv",
                                           bufs=2)
                            for k in range(CT):
                                nc.tensor.matmul(pv[:], xbf[k][:, sl], wv[k][:],
                                                 start=(k == 0), stop=(k == CT - 1))
                            v0i = pA.tile([128, C], bf16, name="v0i", tag="v0i",
                                          bufs=3)
                            nc.scalar.copy(v0i[:], pv[:])
                            kh = pA.tile([128, C], bf16, name="khat", tag="khat",
                                         bufs=4)
                            nc.vector.tensor_scalar(kh[:], k0p[i][:], 1.0,
                                                    e_tall[:, i:i + 1],
                                                    ALU.add, ALU.mult)
                            for k in range(CT):
                                nc.tensor.matmul(psum_kv[k][:],
                                                 kh[:, k * 128:(k + 1) * 128],
                                                 v0i[:], start=(i == 0),
                                                 stop=(i == NSTRIPE - 1))
                            nc.tensor.matmul(psum_ks[:], ones_col[:], kh[:],
                                             start=(i == 0), stop=(i == NSTRIPE - 1))
                        rsec = pA.tile([128, 1], f32, name="rsec")
                        nc.vector.tensor_reduce(rsec[:], e_tall[:], AX.X, ALU.add)
                        alpha_col = pA.tile([128, 1], f32, name="acol")
                        if _PROBE_NOQ7:
                            nc.vector.memset(alpha_col[:], 1.0)
                        else:
                            se11 = pA.tile([1, 1], f32, name="se11")
                            nc.gpsimd.tensor_reduce(se11[:], rsec[:], AX.C, ALU.add)
                            # alpha = N/se, beta = N/(se*sqrt(C)) (kv scale)
                            alpha11 = pA.tile([1, 1], f32, name="alpha11")
                            nc.vector.reciprocal(alpha11[:], se11[:])
                            nc.vector.tensor_scalar(alpha11[:], alpha11[:],
                                                    float(N), None, ALU.mult)
                            nc.gpsimd.partition_broadcast(alpha_col[:], alpha11[:])
                        beta_col = pA.tile([128, 1], f32, name="beta_col")
                        nc.vector.tensor_scalar(beta_col[:], alpha_col[:],
                                                1.0 / float(np.sqrt(C)), None,
                                                ALU.mult)
                        for k in range(CT):
                            stg = pA.tile([128, C], bf16, name="kvstg", tag="kvstg",
                                          bufs=2)
                            nc.vector.tensor_scalar(stg[:], psum_kv[k][:],
                                                    beta_col[:], None, ALU.mult)
                            nc.sync.dma_start(ag_in[:][k * 128:(k + 1) * 128, :],
                                              stg[:])
                        ks_bf = pA.tile([1, C], bf16, name="ks_bf")
                        nc.vector.tensor_scalar(ks_bf[:], psum_ks[:],
                                                alpha_col[0:1, :], None, ALU.mult)
                        nc.sync.dma_start(ag_in[:][C:C + 1, :], ks_bf[:])

                  if "CC" in parts:
                    nc.gpsimd.collective_compute(
                        "AllGather", ALU.bypass,
                        replica_groups=[[0, 1, 2, 3], [4, 5, 6, 7]],
                        ins=[ag_in[:]], outs=[ag_out[:]],
                    )

                # =================================================================
                # PHASE A': phi_first (N-shard) -> DRAM spill  (covers AG latency)
                # =================================================================
                with (
                    tc.tile_pool(name="pA2", bufs=1) as pA2,
                    tc.tile_pool(name="psA2", bufs=1, space="PSUM") as psA2,
                ):
                  if "A2" in parts:
                    for b in range(B):
                        ff = [pA2.tile([128, NL], f32, name="ff", tag="ff", bufs=8)
                              for _ in range(CT)]
                        fbf = [pA2.tile([128, NL], bf16, name="fbf", tag="fbf",
                                        bufs=8) for _ in range(CT)]
                        for k in range(CT):
                            nc.sync.dma_start(ff[k][:],
                                              xs[b, k * 128:(k + 1) * 128, :])
                            nc.vector.tensor_copy(fbf[k][:], ff[k][:])
                        for d in range(CT):
                            pf = psA2.tile([128, NL], f32, name="ppf", tag="ppf",
                                           bufs=2)
                            for k in range(CT):
                                nc.tensor.matmul(pf[:],
                                                 wphi[k][:, d * 128:(d + 1) * 128],
                                                 fbf[k][:], start=(k == 0),
                                                 stop=(k == CT - 1))
                            nc.scalar.activation(phi1[b][d][:], pf[:], AF.Copy,
                                                 scale=1.0 / M)

                # =================================================================
                # PHASE B: q/attention for 12 rest batches on local positions
                # =================================================================
                with (
                    tc.tile_pool(name="pB", bufs=1) as pB,
                    tc.tile_pool(name="psB", bufs=1, space="PSUM") as psB,
                ):
                  if "B" in parts:
                    qbuf = {}

                    def emit_qpr(m):
                        rf = [pB.tile([128, NL], f32, name="rf", tag="rf", bufs=8)
                              for _ in range(CT)]
                        rb = [pB.tile([128, NL], bf16, name="rb", tag="rb", bufs=8)
                              for _ in range(CT)]
                        for k in range(CT):
                            nc.sync.dma_start(rf[k][:],
                                              xs[B + m, k * 128:(k + 1) * 128, :])
                            nc.scalar.copy(rb[k][:], rf[k][:])
                        qb, pb = [], []
                        for g in range(4):
                            pq = psB.tile([128, NL], f32, name="pq", tag="pq", bufs=1)
                            nc.tensor.matmul(pq[:], wq[g][:], rb[g][:], start=True,
                                             stop=True)
                            q = pB.tile([128, NL], bf16, name="qbf", tag="qbf",
                                        bufs=4 * (PREFETCH_M + 2))
                            nc.vector.tensor_scalar(q[:], pq[:], 0.0, 1.0, ALU.max,
                                                    ALU.add)
                            qb.append(q)
                        for d in range(CT):
                            pr = psB.tile([128, NL], f32, name="ppr", tag="ppr",
                                          bufs=2)
                            for k in range(CT):
                                nc.tensor.matmul(pr[:],
                                                 wphi[k][:, d * 128:(d + 1) * 128],
                                                 rb[k][:], start=(k == 0),
                                                 stop=(k == CT - 1))
                            p = pB.tile([128, NL], bf16, name="prbf", tag="prbf",
                                        bufs=4 * (PREFETCH_M + 2))
                            nc.scalar.copy(p[:], pr[:])
                            pb.append(p)
                        qbuf[m] = (qb, pb)

                    def emit_attn(m):
                        b = m % B
                        qb, pb = qbuf.pop(m)
                        pz = psB.tile([1, NL], f32, name="pz", tag="pz", bufs=1)
                        for k in range(CT):
                            nc.tensor.matmul(pz[:], kscol[b][k][:], qb[k][:],
                                             start=(k == 0), stop=(k == CT - 1))
                        zrow = pB.tile([1, NL], f32, name="zrow", tag="zrow", bufs=2)
                        nc.vector.tensor_scalar(zrow[:], pz[:], EPS_Z, None, ALU.add)
                        nc.vector.reciprocal(zrow[:], zrow[:])
                        zrow_bf = pB.tile([1, NL], bf16, name="zrow_bf",
                                          tag="zrow_bf", bufs=2)
                        nc.scalar.copy(zrow_bf[:], zrow[:])
                        zb_ps = psB.tile([128, NL], f32, name="zb_ps", tag="zb_ps",
                                         bufs=1)
                        nc.tensor.matmul(zb_ps[:], ones_row_b[:], zrow_bf[:],
                                         start=True, stop=True)
                        zbc = pB.tile([128, NL], bf16, name="zbc", tag="zbc", bufs=2)
                        nc.scalar.copy(zbc[:], zb_ps[:])
                        qz = []
                        for k in range(CT):
                            t = pB.tile([128, NL], bf16, name="qz", tag="qz", bufs=8)
                            nc.vector.tensor_tensor(t[:], qb[k][:], zbc[:], ALU.mult)
                            qz.append(t)
                        for d in range(CT):
                            sp = psB.tile([128, NL], f32, name="psmm", tag="psmm",
                                          bufs=2)
                            for k in range(CT):
                                nc.tensor.matmul(sp[:],
                                                 kvb[b][k][:, d * 128:(d + 1) * 128],
                                                 qz[k][:], start=(k == 0),
                                                 stop=(k == CT - 1))
                            tmp = pB.tile([128, NL], f32, name="ytmp", tag="ytmp",
                                          bufs=2)
                            nc.vector.tensor_tensor(tmp[:], sp[:], pb[d][:], ALU.mult)
                            nc.vector.tensor_tensor(ysum[d][:], ysum[d][:], tmp[:],
                                                    ALU.add)

                    for m in range(PREFETCH_M):
                        emit_qpr(m)
                    # post-AG loads (kv + ksum for all 4 batches)
                    ago = ag_out[:]
                    for b in range(B):
                        for k in range(CT):
                            nc.sync.dma_start(
                                kvb[b][k][:],
                                ago[b * (C + 1) + k * 128:
                                    b * (C + 1) + (k + 1) * 128, :])
                            nc.sync.dma_start(
                                kscol[b][k][:],
                                ago[b * (C + 1) + C: b * (C + 1) + C + 1,
                                    k * 128:(k + 1) * 128].rearrange("a b -> b a"))
                    for m in range(M):
                        if m >= PREFETCH_M:
                            emit_qpr(m)
                        emit_attn(m)

                # =================================================================
                # PHASE C: y_first, LN1, FFN, LN2, relu  (per output batch b)
                # =================================================================
                with (
                    tc.tile_pool(name="pC", bufs=1) as pC,
                    tc.tile_pool(name="psC", bufs=1, space="PSUM") as psC,
                ):
                  if "C" in parts:
                    for b in range(B):
                        fst = [pC.tile([128, NL], f32, name="fst", tag="fst", bufs=6)
                               for _ in range(CT)]
                        for k in range(CT):
                            nc.sync.dma_start(fst[k][:],
                                              xs[b, k * 128:(k + 1) * 128, :])
                        res1 = [pC.tile([128, NL], f32, name=f"res1_{k}",
                                        tag=f"res1_{k}", bufs=1) for k in range(CT)]
                        for k in range(CT):
                            nc.vector.tensor_tensor(res1[k][:], ysum[k][:],
                                                    phi1[b][k][:], ALU.mult)
                            nc.vector.tensor_tensor(res1[k][:], res1[k][:],
                                                    fst[k][:], ALU.add)

                        def layer_norm(xtiles, out_cb):
                            # stats over channel (partition) axis via PE ones-matmul
                            xbt = [pC.tile([128, NL], bf16, name="lnxb", tag="lnxb",
                                           bufs=8) for _ in range(CT)]
                            sqt = [pC.tile([128, NL], bf16, name="lnsqb",
                                           tag="lnsqb", bufs=8) for _ in range(CT)]
                            for k in range(CT):
                                nc.scalar.copy(xbt[k][:], xtiles[k][:])
                                nc.scalar.activation(sqt[k][:], xtiles[k][:],
                                                     AF.Square)
                            ps_st = psC.tile([1, NL], f32, name="lnst", tag="lnst",
                                             bufs=1)
                            for k in range(CT):
                                nc.tensor.matmul(ps_st[:], ones_col[:], xbt[k][:],
                                                 start=(k == 0), stop=(k == CT - 1))
                            mu = pC.tile([1, NL], f32, name="lnmu", tag="lnmu",
                                         bufs=1)
                            nc.scalar.activation(mu[:], ps_st[:], AF.Copy,
                                                 scale=1.0 / C)
                            ps_sq = psC.tile([1, NL], f32, name="lnst", tag="lnst",
                                             bufs=1)
                            for k in range(CT):
                                nc.tensor.matmul(ps_sq[:], ones_col[:], sqt[k][:],
                                                 start=(k == 0), stop=(k == CT - 1))
                            musq = pC.tile([1, NL], f32, name="lnmusq", tag="lnmusq",
                                           bufs=1)
                            nc.scalar.activation(musq[:], mu[:], AF.Square)
                            var = pC.tile([1, NL], f32, name="lnvar", tag="lnvar",
                                          bufs=1)
                            nc.vector.scalar_tensor_tensor(
                                var[:], ps_sq[:], 1.0 / C, musq[:],
                                ALU.mult, ALU.subtract)
                            sd = pC.tile([1, NL], f32, name="lnsd", tag="lnsd",
                                         bufs=1)
                            nc.scalar.activation(sd[:], var[:], AF.Sqrt,
                                                 bias=eps_ln1[:])
                            arow = pC.tile([1, NL], f32, name="lnA", tag="lnA",
                                           bufs=1)
                            nc.vector.reciprocal(arow[:], sd[:])
                            brow = pC.tile([1, NL], f32, name="lnB", tag="lnB",
                                           bufs=1)
                            nc.vector.scalar_tensor_tensor(
                                brow[:], mu[:], -1.0, arow[:], ALU.mult, ALU.mult)
                            # broadcast a/b rows to 128 partitions via PE (f32)
                            ps_a = psC.tile([128, NL], f32, name="lnbc", tag="lnbc",
                                            bufs=1)
                            nc.tensor.matmul(ps_a[:], ones_row_f[:], arow[:],
                                             start=True, stop=True)
                            abc = pC.tile([128, NL], f32, name="lnAbc", tag="lnAbc",
                                          bufs=1)
                            nc.scalar.copy(abc[:], ps_a[:])
                            ps_b = psC.tile([128, NL], f32, name="lnbc", tag="lnbc",
                                            bufs=1)
                            nc.tensor.matmul(ps_b[:], ones_row_f[:], brow[:],
                                             start=True, stop=True)
                            bbc = pC.tile([128, NL], f32, name="lnBbc", tag="lnBbc",
                                          bufs=1)
                            nc.scalar.copy(bbc[:], ps_b[:])
                            for k in range(CT):
                                t = pC.tile([128, NL], f32, name="lnt", tag="lnt",
                                            bufs=2)
                                nc.vector.tensor_tensor(t[:], xtiles[k][:], abc[:],
                                                        ALU.mult)
                                nc.vector.tensor_tensor(t[:], t[:], bbc[:], ALU.add)
                                out_cb(k, t)

                        o1 = [pC.tile([128, NL], f32, name=f"o1_{k}",
                                      tag=f"o1_{k}", bufs=1) for k in range(CT)]
                        o1b = [pC.tile([128, NL], bf16, name=f"o1b_{k}",
                                       tag=f"o1b_{k}", bufs=1) for k in range(CT)]

                        def ln1_out(k, t):
                            nc.scalar.activation(o1[k][:], t[:], AF.Identity,
                                                 bias=ln1b[:, k:k + 1],
                                                 scale=ln1w[:, k:k + 1])
                            nc.vector.tensor_copy(o1b[k][:], o1[k][:])

                        layer_norm(res1, ln1_out)

                        # FFN: h = relu(fc1@o1+b1) [j,n]; o = fc2@h [d,n]
                        po = [psC.tile([128, NL], f32, name=f"po{d}", tag=f"po{d}",
                                       bufs=1) for d in range(CT)]
                        for j in range(JT):
                            ph = psC.tile([128, NL], f32, name="ph", tag="ph",
                                          bufs=2)
                            for k in range(CT):
                                nc.tensor.matmul(ph[:],
                                                 fc1[k][:, j * 128:(j + 1) * 128],
                                                 o1b[k][:], start=(k == 0),
                                                 stop=(k == CT - 1))
                            hb = pC.tile([128, NL], bf16, name="hbf", tag="hbf",
                                         bufs=3)
                            nc.scalar.activation(hb[:], ph[:], AF.Relu,
                                                 bias=fc1b[:, j:j + 1])
                            for d in range(CT):
                                nc.tensor.matmul(po[d][:],
                                                 fc2[j][:, d * 128:(d + 1) * 128],
                                                 hb[:], start=(j == 0),
                                                 stop=(j == JT - 1))
                        o2 = [pC.tile([128, NL], f32, name=f"o2_{k}",
                                      tag=f"o2_{k}", bufs=1) for k in range(CT)]
                        for d in range(CT):
                            nc.scalar.activation(o2[d][:], po[d][:], AF.Identity,
                                                 bias=fc2b[:, d:d + 1])
                            nc.vector.tensor_tensor(o2[d][:], o2[d][:], o1[d][:],
                                                    ALU.add)

                        def ln2_out(k, t):
                            ot = pC.tile([128, NL], f32, name="otile", tag="otile",
                                         bufs=4)
                            nc.scalar.activation(ot[:], t[:], AF.Relu,
                                                 bias=ln2b[:, k:k + 1],
                                                 scale=ln2w[:, k:k + 1])
                            nc.sync.dma_start(
                                out_d.ap()[b, k * 128:(k + 1) * 128, :], ot[:])

                        layer_norm(o2, ln2_out)

    nc.compile()
    return nc


def _prep_inputs(inputs):
    x = np.asarray(inputs['x'], np.float32)
    bf = ml_dtypes.bfloat16

    def col(a):   # [k*128] -> [128, k]
        a = np.asarray(a, np.float32)
        return np.ascontiguousarray(a.reshape(-1, 128).T)

    shared = {
        "wqT": np.ascontiguousarray(
            np.asarray(inputs['Wq'], np.float32).transpose(0, 2, 1)).astype(bf),
        "wkT": np.ascontiguousarray(
            np.asarray(inputs['Wk'], np.float32).transpose(0, 2, 1)).astype(bf),
        "wvT": np.ascontiguousarray(np.asarray(inputs['Wv'], np.float32).T).astype(bf),
        "wphiT": np.ascontiguousarray(
            np.asarray(inputs['Wphi'], np.float32).T).astype(bf),
        "fc1T": np.ascontiguousarray(
            np.asarray(inputs['fc1_w'], np.float32).T).astype(bf),
        "fc2T": np.ascontiguousarray(
            np.asarray(inputs['fc2_w'], np.float32).T).astype(bf),
        "fc1b": col(inputs['fc1_b']),
        "fc2b": col(inputs['fc2_b']),
        "ln1w": col(inputs['ln1_w']),
        "ln1b": col(inputs['ln1_b']),
        "ln2w": col(inputs['ln2_w']),
        "ln2b": col(inputs['ln2_b']),
    }
    in_maps = []
    for c in range(NCORES):
        m = dict(shared)
        m["xs"] = np.ascontiguousarray(x[:, :, c * NL:(c + 1) * NL])
        m["xfull"] = np.ascontiguousarray(x[c % B])
        in_maps.append(m)
    return in_maps


def kernel(**inputs):
    if "nc" not in _CACHE:
        _CACHE["nc"] = _build()
    nc = _CACHE["nc"]
    in_maps = _prep_inputs(inputs)
    r = bass_utils.run_bass_kernel_spmd(nc, in_maps, core_ids=list(range(NCORES)))
    x = np.asarray(inputs['x'], np.float32)
    out_first = np.empty((B, C, N), np.float32)
    for c in range(NCORES):
        out_first[:, :, c * NL:(c + 1) * NL] = r.results[c]["out"]
    return np.concatenate([out_first, x[B:]], axis=0)

